# revision 3
# baseline (speedup 1.0000x reference)
"""Grouped GEMM (MoE routing) Trainium2 kernel.

Expert-parallel across 8 NeuronCores with size-sorted slot assignment:
experts are sorted by token count and slot s on every core holds the
experts of size-rank [8s, 8s+8), so one SPMD program with per-slot
capacities cap_s = roundup4(max count in rank group) serves all cores.

Flipped matmul orientation: weights are the stationary operand
([128 K, 128 DOUT] fp8 e3m4 blocks, per-expert scale undone on host)
and token tiles stream as the moving operand ([128 K, cap] bf16), so
PE stream time is proportional to actual routed tokens instead of
ceil(cap/128) full 128-lane tiles. Output leaves transposed
([13, 128, sumcap] bf16) and is unpacked on host.
"""
import ml_dtypes
import numpy as np

import concourse.bass as bass
import concourse.mybir as mybir
import concourse.tile as tile
from concourse import bacc
from concourse.bass_utils import run_bass_kernel_spmd

G, T, DIN, DOUT = 64, 8192, 2560, 1664
NCORES = 8
EPC = G // NCORES   # expert slots per core
KC = DIN // 128     # 20 contraction chunks
OC = DOUT // 128    # 13 output-row chunks
W8MAX = 15.0        # e3m4 scale target (max normal 15.5)

_cache = {}


def _build(caps):
    offs = np.concatenate([[0], np.cumsum(caps)]).astype(int)
    sumcap = int(offs[-1])
    nc = bacc.Bacc(trn_type="TRN2", debug=False)
    bf16 = mybir.dt.bfloat16
    e3 = mybir.dt.float8e3
    xt = nc.dram_tensor("xt", [128, KC * sumcap], bf16, kind="ExternalInput").ap()
    w = nc.dram_tensor("w", [EPC, 128, KC * DOUT], e3, kind="ExternalInput").ap()
    out = nc.dram_tensor(
        "out", [OC, 128, sumcap], bf16, kind="ExternalOutput"
    ).ap()
    with tile.TileContext(nc) as tc:
        with (
            tc.tile_pool(name="xtp", bufs=3) as xt_pool,
            tc.tile_pool(name="wp", bufs=2) as w_pool,
            tc.tile_pool(name="op", bufs=6) as o_pool,
            tc.tile_pool(name="ps", bufs=1, space="PSUM") as ps_pool,
        ):
            for s in range(EPC):
                cap = int(caps[s])
                if cap == 0:
                    continue
                off = int(offs[s])
                xt_sb = xt_pool.tile([128, KC * cap], bf16, tag="xt", name=f"xt{s}")
                nc.gpsimd.dma_start(
                    xt_sb[:], xt[:, KC * off:KC * (off + cap)]
                )
                w_sb = w_pool.tile([128, KC * DOUT], e3, tag="w", name=f"w{s}")
                nc.sync.dma_start(w_sb[:], w[s])
                for oc in range(OC):
                    psum = ps_pool.tile(
                        [128, cap], mybir.dt.float32, tag=f"ps{oc % 2}",
                        name=f"psum_{s}_{oc}",
                    )
                    for k in range(KC):
                        nc.tensor.matmul(
                            psum[:],
                            w_sb[:, k * DOUT + oc * 128: k * DOUT + oc * 128 + 128],
                            xt_sb[:, k * cap:(k + 1) * cap],
                            start=(k == 0),
                            stop=(k == KC - 1),
                        )
                    o_sb = o_pool.tile([128, cap], bf16, tag="o",
                                       name=f"o_{s}_{oc}")
                    nc.vector.tensor_copy(o_sb[:], psum[:])
                    nc.scalar.dma_start(out[oc, :, off:off + cap], o_sb[:])
    nc.compile()
    return nc


def _run(inputs, trace=False):
    x = np.asarray(inputs["input"], dtype=np.float32)
    w = np.ascontiguousarray(np.asarray(inputs["weight"], dtype=np.float32))
    counts = np.asarray(inputs["tokens_per_expert"], dtype=np.int64)
    starts = np.concatenate([[0], np.cumsum(counts)[:-1]])

    order = np.argsort(-counts, kind="stable")  # experts by size rank
    # slot s, core c -> expert order[s*NCORES + c]; capacity = rank-group max
    caps = tuple(
        int(np.ceil(max(1, counts[order[s * NCORES:(s + 1) * NCORES]].max()) / 4) * 4)
        for s in range(EPC)
    )
    offs = np.concatenate([[0], np.cumsum(caps)]).astype(int)
    sumcap = int(offs[-1])

    if caps not in _cache:
        _cache[caps] = _build(caps)
    nc = _cache[caps]

    # per-expert fp8 scale
    wmax = np.abs(w).max(axis=(1, 2))
    alpha = np.where(wmax > 0, W8MAX / np.maximum(wmax, 1e-30), 1.0)

    in_maps = []
    for c in range(NCORES):
        xt_pack = np.zeros((128, KC * sumcap), dtype=ml_dtypes.bfloat16)
        w_pack = np.empty((EPC, 128, KC * DOUT), dtype=ml_dtypes.float8_e3m4)
        for s in range(EPC):
            g = int(order[s * NCORES + c])
            cnt = int(counts[g])
            cap = caps[s]
            if cnt:
                # [cnt, DIN] -> [128, KC, cnt] (partition, k-chunk, token)
                xs = x[starts[g]:starts[g] + cnt].T.reshape(KC, 128, cnt)
                xt_pack[:, KC * offs[s]:KC * (offs[s] + cap)] \
                    .reshape(128, KC, cap)[:, :, :cnt] = xs.swapaxes(0, 1)
            wq = (w[g] * alpha[g]).astype(ml_dtypes.float8_e3m4)
            w_pack[s] = wq.reshape(KC, 128, DOUT).swapaxes(0, 1) \
                .reshape(128, KC * DOUT)
        in_maps.append({"xt": xt_pack, "w": w_pack})

    kw = {"trace_cores": list(range(NCORES))} if trace else {}
    res = run_bass_kernel_spmd(nc, in_maps, core_ids=list(range(NCORES)),
                               trace=trace, **kw)

    out = np.empty((T, DOUT), dtype=np.float32)
    for c in range(NCORES):
        for s in range(EPC):
            g = int(order[s * NCORES + c])
            cnt = int(counts[g])
            if cnt:
                # [OC, 128, cnt] -> [cnt, DOUT]
                y = res.results[c]["out"][:, :, offs[s]:offs[s] + cnt]
                out[starts[g]:starts[g] + cnt] = \
                    y.transpose(2, 0, 1).reshape(cnt, DOUT).astype(np.float32) \
                    * (1.0 / alpha[g])
    return out, res


def kernel(**inputs) -> np.ndarray:
    return _run(inputs)[0]


# revision 5
# speedup vs baseline: 1.1981x; 1.1981x over previous
"""Grouped GEMM (MoE routing) Trainium2 kernel.

Expert-parallel across 8 NeuronCores with size-sorted slot assignment:
experts are sorted by token count and slot s on every core holds the
experts of size-rank [8s, 8s+8), so one SPMD program with per-slot
capacities cap_s = roundup4(max count in rank group) serves all cores.

Flipped matmul orientation: weights are the stationary operand
([128 K, 128 DOUT] fp8 e3m4 blocks, per-expert scale undone on host)
and token tiles stream as the moving operand ([128 K, cap] bf16), so
PE stream time is proportional to actual routed tokens instead of
ceil(cap/128) full 128-lane tiles. Output leaves transposed
([13, 128, sumcap] bf16) and is unpacked on host.
"""
import ml_dtypes
import numpy as np

import concourse.bass as bass
import concourse.mybir as mybir
import concourse.tile as tile
from concourse import bacc
from concourse.bass_utils import run_bass_kernel_spmd

G, T, DIN, DOUT = 64, 8192, 2560, 1664
NCORES = 8
EPC = G // NCORES   # expert slots per core
KC = DIN // 128     # 20 contraction chunks
OC = DOUT // 128    # 13 output-row chunks
W8MAX = 15.0        # e3m4 scale target (max normal 15.5)

_cache = {}


def _build(caps):
    offs = np.concatenate([[0], np.cumsum(caps)]).astype(int)
    sumcap = int(offs[-1])
    nc = bacc.Bacc(trn_type="TRN2", debug=False)
    bf16 = mybir.dt.bfloat16
    e3 = mybir.dt.float8e3
    xt = nc.dram_tensor("xt", [128, KC * sumcap], bf16, kind="ExternalInput").ap()
    w = nc.dram_tensor("w", [EPC, KC, 128, DOUT], e3, kind="ExternalInput").ap()
    out = nc.dram_tensor(
        "out", [OC, 128, sumcap], bf16, kind="ExternalOutput"
    ).ap()
    with tile.TileContext(nc) as tc:
        with (
            tc.tile_pool(name="xtp", bufs=3) as xt_pool,
            tc.tile_pool(name="wp", bufs=44) as w_pool,
            tc.tile_pool(name="op", bufs=6) as o_pool,
            tc.tile_pool(name="ps", bufs=1, space="PSUM") as ps_pool,
        ):
            for s in range(EPC):
                cap = int(caps[s])
                if cap == 0:
                    continue
                off = int(offs[s])
                xt_sb = xt_pool.tile([128, KC * cap], bf16, tag="xt", name=f"xt{s}")
                nc.gpsimd.dma_start(
                    xt_sb[:], xt[:, KC * off:KC * (off + cap)]
                )
                w_sb = {}
                for k in range(KC):
                    w_sb[k] = w_pool.tile([128, DOUT], e3, tag="w",
                                          name=f"w{s}_{k}")
                    nc.sync.dma_start(w_sb[k][:], w[s, k])
                for oc in range(OC):
                    psum = ps_pool.tile(
                        [128, cap], mybir.dt.float32, tag=f"ps{oc % 2}",
                        name=f"psum_{s}_{oc}",
                    )
                    for k in range(KC):
                        nc.tensor.matmul(
                            psum[:],
                            w_sb[k][:, oc * 128:oc * 128 + 128],
                            xt_sb[:, k * cap:(k + 1) * cap],
                            start=(k == 0),
                            stop=(k == KC - 1),
                        )
                    o_sb = o_pool.tile([128, cap], bf16, tag="o",
                                       name=f"o_{s}_{oc}")
                    nc.vector.tensor_copy(o_sb[:], psum[:])
                    nc.scalar.dma_start(out[oc, :, off:off + cap], o_sb[:])
    nc.compile()
    return nc


def _run(inputs, trace=False):
    x = np.asarray(inputs["input"], dtype=np.float32)
    w = np.ascontiguousarray(np.asarray(inputs["weight"], dtype=np.float32))
    counts = np.asarray(inputs["tokens_per_expert"], dtype=np.int64)
    starts = np.concatenate([[0], np.cumsum(counts)[:-1]])

    order = np.argsort(-counts, kind="stable")  # experts by size rank
    # slot s, core c -> expert order[s*NCORES + c]; capacity = rank-group max
    caps = tuple(
        int(np.ceil(max(1, counts[order[s * NCORES:(s + 1) * NCORES]].max()) / 4) * 4)
        for s in range(EPC)
    )
    offs = np.concatenate([[0], np.cumsum(caps)]).astype(int)
    sumcap = int(offs[-1])

    if caps not in _cache:
        _cache[caps] = _build(caps)
    nc = _cache[caps]

    # per-expert fp8 scale
    wmax = np.abs(w).max(axis=(1, 2))
    alpha = np.where(wmax > 0, W8MAX / np.maximum(wmax, 1e-30), 1.0)

    in_maps = []
    for c in range(NCORES):
        xt_pack = np.zeros((128, KC * sumcap), dtype=ml_dtypes.bfloat16)
        w_pack = np.empty((EPC, KC, 128, DOUT), dtype=ml_dtypes.float8_e3m4)
        for s in range(EPC):
            g = int(order[s * NCORES + c])
            cnt = int(counts[g])
            cap = caps[s]
            if cnt:
                # [cnt, DIN] -> [128, KC, cnt] (partition, k-chunk, token)
                xs = x[starts[g]:starts[g] + cnt].T.reshape(KC, 128, cnt)
                xt_pack[:, KC * offs[s]:KC * (offs[s] + cap)] \
                    .reshape(128, KC, cap)[:, :, :cnt] = xs.swapaxes(0, 1)
            w_pack[s] = (w[g] * alpha[g]).astype(ml_dtypes.float8_e3m4) \
                .reshape(KC, 128, DOUT)
        in_maps.append({"xt": xt_pack, "w": w_pack})

    kw = {"trace_cores": list(range(NCORES))} if trace else {}
    res = run_bass_kernel_spmd(nc, in_maps, core_ids=list(range(NCORES)),
                               trace=trace, **kw)

    out = np.empty((T, DOUT), dtype=np.float32)
    for c in range(NCORES):
        for s in range(EPC):
            g = int(order[s * NCORES + c])
            cnt = int(counts[g])
            if cnt:
                # [OC, 128, cnt] -> [cnt, DOUT]
                y = res.results[c]["out"][:, :, offs[s]:offs[s] + cnt]
                out[starts[g]:starts[g] + cnt] = \
                    y.transpose(2, 0, 1).reshape(cnt, DOUT).astype(np.float32) \
                    * (1.0 / alpha[g])
    return out, res


def kernel(**inputs) -> np.ndarray:
    return _run(inputs)[0]
